# revision 2
# baseline (speedup 1.0000x reference)
"""Local sparse (banded) attention for Trainium2, 8 NeuronCores — v4.

Problem: B=2, H=12, L=4096, D=64, window=128 (position i attends [i-128, i+128]).

Structure (per core, 3 heads, fully pipelined):
- 128-wide key blocks; QK pass t computes blocks (t, t+16) as two concurrent
  PE row-group matmuls (kt/qt split across partition halves, 320-col halo on
  qt). All heads pair within themselves — no half-idle solo pass.
- exp on ScalarE per pass: [128, 2, 384] PSUM -> SBUF fp16.
- Band masks as DVE fp16 multiplies batched over 2-pass groups (big ops keep
  the 2x DVE mode effective); the two full edge masks run on GpSimd.
- PV: per query block, 3 accumulating C=128 matmuls (N=65, ones-column
  carries the softmax denominator). Query blocks are emitted as soon as
  their pt tiles are masked: block i (lo half) after mask batch (i+1)//2,
  block 17..30 (hi half) consume early passes, so PE work is spread evenly
  across the pass loop instead of bursting at the tail. og tiles group
  {2k, 2k+1, 2k+17, 2k+18} (affine normalize APs); tail {14,15} & {16,31}.
- First head's inputs DMA in 4 chunks so compute starts ~2us in; later
  heads load with one DMA per tensor (SP issue time off the critical path).
- fp16 output, host casts to fp32.
"""

import os
import sys

sys.path.insert(0, "/opt/trn_rl_repo")
os.environ.setdefault("JAX_PLATFORMS", "axon")

import numpy as np

import concourse.bass as bass
import concourse.mybir as mybir
from concourse import tile

B, H, L, D = 2, 12, 4096, 64
W = 128
NBLK = 32
NP = 16
HPC = 3
NCORES = 8
F16 = mybir.dt.float16
F32 = mybir.dt.float32
EXP = mybir.ActivationFunctionType.Exp

WINS = [0 if j == 0 else (L - 384 if j == NBLK - 1 else (j - 1) * 128)
        for j in range(NBLK)]


def _rel_slice(i: int, j: int) -> int:
    if j == 0:
        return i
    if j == NBLK - 1:
        return i - (NBLK - 3)
    return i - j + 1


_NO_SPLIT_OPCODES = {"AllEngineBarrier", "Halt", "Call", "Branch",
                     "CompareAndBranch", "IndirectBranch", "BranchHint"}


def _legalize_matmul_waits(nc: bass.Bass) -> None:
    """TPB engine instructions encode a single sync wait; walrus refuses
    more. Split extras onto NoOps (one wait each) inserted right before the
    instruction on the same engine queue."""
    f = nc.m.functions[0]
    for blk in f.blocks:
        il = blk.instructions
        idx = 0
        while idx < len(il):
            inst = il[idx]
            si = inst.sync_info
            if (
                si is not None
                and len(si.on_wait) > 1
                and inst.opcode not in _NO_SPLIT_OPCODES
            ):
                waits = list(si.on_wait)
                for w_i, w in enumerate(waits[:-1]):
                    nop = mybir.InstNoOp(name=f"{inst.name}-wnop{w_i}")
                    nop.engine = inst.engine
                    nop.sync_info = mybir.SyncInfo(on_wait=[w], on_update=[])
                    nc.register_instruction(nop)
                    il.insert(idx, nop)
                    idx += 1
                inst.sync_info = mybir.SyncInfo(
                    on_wait=waits[-1:], on_update=list(si.on_update)
                )
            idx += 1


# og tile k -> its 4 (or 2) query blocks; chosen so each tile's blocks become
# ready within one 2-pass window and the normalize APs stay affine
OG_TILES = [[2 * k, 2 * k + 1, 2 * k + 17, 2 * k + 18] for k in range(7)] + \
           [[14, 15], [16, 31]]
# query block -> (og tile index, slot)
_BLK2OG = {}
for _k, _blks in enumerate(OG_TILES):
    for _m, _b in enumerate(_blks):
        _BLK2OG[_b] = (_k, _m)


def _ready_pass(i: int) -> int:
    """first pass t after whose mask batch block i's PV can run (2-pass mask
    batches complete at odd t)"""
    if i in (14, 15, 16, 31):
        return 15
    need = i + 1 if i <= 13 else i - 15   # highest pass whose pt it reads
    return need if need % 2 == 1 else need + 1


BLOCKS_AT: dict = {}
for _i in range(NBLK):
    BLOCKS_AT.setdefault(_ready_pass(_i), []).append(_i)
NORMS_AT: dict = {}
for _k, _blks in enumerate(OG_TILES):
    NORMS_AT.setdefault(max(_ready_pass(b) for b in _blks), []).append(_k)


def build_nc(n_heads: int = HPC, repeat: int = 1) -> bass.Bass:
    nc = bass.Bass("TRN2", target_bir_lowering=False, debug=False)
    qh = nc.dram_tensor("qh", [n_heads, 2, 64, 2176], F16, kind="ExternalInput").ap()
    kh = nc.dram_tensor("kh", [n_heads, 2, 64, 2048], F16, kind="ExternalInput").ap()
    vA = nc.dram_tensor("vA", [n_heads, 128, NBLK, 65], F16, kind="ExternalInput").ap()
    mO = nc.dram_tensor("mO", [128, 2, 128], F16, kind="ExternalInput").ap()
    mF = nc.dram_tensor("mF", [128, 384], F16, kind="ExternalInput").ap()
    mL = nc.dram_tensor("mL", [128, 384], F16, kind="ExternalInput").ap()
    out = nc.dram_tensor("out", [n_heads, NBLK, 128, 64], F16, kind="ExternalOutput").ap()

    with tile.TileContext(nc) as tc:
        with (
            tc.tile_pool(name="cst", bufs=1) as cst,
            tc.tile_pool(name="io", bufs=2) as io,
            tc.tile_pool(name="pss", bufs=3, space="PSUM") as pss,
            tc.tile_pool(name="pso", bufs=2, space="PSUM") as pso,
        ):
            mask_o = cst.tile([128, 2 * 128], F16, name="mask_o")
            mask_f = cst.tile([128, 384], F16, name="mask_f")
            mask_l = cst.tile([128, 384], F16, name="mask_l")
            nc.sync.dma_start(out=mask_o.rearrange("p (s x) -> p s x", s=2), in_=mO)
            nc.sync.dma_start(out=mask_f, in_=mF)
            nc.sync.dma_start(out=mask_l, in_=mL)
            mv = mask_o.rearrange("p (s x) -> p s x", s=2)

            for rh in range(repeat * n_heads):
                h = rh % n_heads
                qt = io.tile([128, 2176], F16, tag="qt", name=f"qt{rh}")
                kt = io.tile([128, 2048], F16, tag="kt", name=f"kt{rh}")
                vt = io.tile([128, NBLK * 65], F16, tag="vt", name=f"vt{rh}")
                qf = qh[h].rearrange("u p c -> (u p) c")
                kf = kh[h].rearrange("u p c -> (u p) c")
                if rh == 0:
                    vv = vt.rearrange("p (m c) -> p m c", m=NBLK)
                    for ci in range(4):
                        nc.sync.dma_start(
                            out=qt[:, 544 * ci:544 * ci + 544],
                            in_=qf[:, 544 * ci:544 * ci + 544])
                        nc.sync.dma_start(
                            out=kt[:, 512 * ci:512 * ci + 512],
                            in_=kf[:, 512 * ci:512 * ci + 512])
                        if ci < 2:
                            nc.sync.dma_start(
                                out=vv[:, 16 * ci:16 * ci + 16, :],
                                in_=vA[h][:, 16 * ci:16 * ci + 16, :])
                else:
                    nc.sync.dma_start(out=qt, in_=qf)
                    nc.sync.dma_start(out=kt, in_=kf)
                    nc.sync.dma_start(
                        out=vt.rearrange("p (m c) -> p m c", m=NBLK), in_=vA[h])

                # pt layout: [128, t(16), u(2), 384] fp16
                pt = io.tile([128, NP * 768], F16, tag="pt", name=f"pt{rh}")
                ot = io.tile([128, NBLK * 64], F16, tag="ot", name=f"ot{rh}")

                def emit_mask_batch(w):
                    """mask pt for passes 2w, 2w+1 — one big DVE op per
                    outer-slice pair; edge tiles get full-tile masks."""
                    t0 = 2 * w
                    # merged (t, u) dim: [128, (2 passes * 2 halves), 3, 128]
                    base = pt[:, 768 * t0:768 * (t0 + 2)].rearrange(
                        "p (n x) -> p n x", n=4).rearrange(
                        "p n (s x) -> p n s x", s=3)
                    if w == 0:
                        sl = base[:, 1:4, 0:3:2, :]   # skip (t=0,u=0) edge tile
                        nc.gpsimd.tensor_tensor(
                            pt[:, 0:384], pt[:, 0:384], mask_f,
                            mybir.AluOpType.mult)
                    elif w == 7:
                        sl = base[:, 0:3, 0:3:2, :]   # skip (t=15,u=1) edge tile
                        nc.gpsimd.tensor_tensor(
                            pt[:, 768 * 15 + 384:768 * 16],
                            pt[:, 768 * 15 + 384:768 * 16], mask_l,
                            mybir.AluOpType.mult)
                    else:
                        sl = base[:, :, 0:3:2, :]
                    m = mask_o.rearrange("p (s x) -> p s x", s=2)[
                        :, None, :, :].to_broadcast(sl.shape)
                    nc.vector.tensor_tensor(sl, sl, m, mybir.AluOpType.mult)

                og_tiles = {}

                def emit_block_pv(i):
                    k, m = _BLK2OG[i]
                    if k not in og_tiles:
                        og_tiles[k] = [pso.tile(
                            [128, 4 * 65], F32, tag="og", name=f"og{rh}_{k}"), True]
                    og, first = og_tiles[k]
                    js = [jj for jj in (i - 1, i, i + 1) if 0 <= jj < NBLK]
                    for jj in js:
                        tt, uu = jj % 16, jj // 16
                        rel = _rel_slice(i, jj)
                        col = 768 * tt + 384 * uu + 128 * rel
                        nc.tensor.matmul(
                            og[:, m * 65:m * 65 + 65],
                            lhsT=pt[:, col:col + 128],
                            rhs=vt[:, jj * 65:(jj + 1) * 65],
                            start=first,
                            stop=(jj == js[-1]),
                            skip_group_check=True,
                        )
                        first = False
                    og_tiles[k][1] = False

                def emit_og_norm(k):
                    og, _ = og_tiles.pop(k)
                    nb = len(OG_TILES[k])
                    ogv = og.rearrange("p (m c) -> p m c", m=4)
                    rg = cst.tile([128, 4], F32, tag="rg", name=f"rg{rh}_{k}", bufs=2)
                    nc.vector.reciprocal(rg[:, 0:nb], ogv[:, 0:nb, 64])
                    av = ot.rearrange("p (g d) -> p g d", g=NBLK)
                    if k < 7:
                        # blocks {2k, 2k+1} and {2k+17, 2k+18}
                        for b0, m0 in ((2 * k, 0), (2 * k + 17, 2)):
                            dd = av[:, b0:b0 + 2, :]
                            nc.vector.tensor_tensor(
                                dd, ogv[:, m0:m0 + 2, 0:64],
                                rg[:, m0:m0 + 2, None].to_broadcast(dd.shape),
                                mybir.AluOpType.mult)
                    else:
                        dd = av[:, 14:16, :] if k == 7 else av[:, 16:32:15, :]
                        nc.vector.tensor_tensor(
                            dd, ogv[:, 0:nb, 0:64],
                            rg[:, 0:nb, None].to_broadcast(dd.shape),
                            mybir.AluOpType.mult)

                for t in range(NP):
                    s2 = pss.tile([128, 1024], F32, tag="s2", name=f"s2_{rh}_{t}")
                    for u, j in enumerate((t, t + 16)):
                        nc.tensor.matmul(
                            s2[:, u * 512:u * 512 + 384],
                            lhsT=kt[u * 64:u * 64 + 64, 128 * t:128 * t + 128],
                            rhs=qt[u * 64:u * 64 + 64,
                                   WINS[j] - 1920 * u:WINS[j] - 1920 * u + 384],
                            start=True, stop=True,
                            tile_position=(u * 64, 0),
                        )
                    dst = pt[:, 768 * t:768 * (t + 1)].rearrange("p (u x) -> p u x", u=2)
                    src = s2.rearrange("p (u x) -> p u x", u=2)[:, :, 0:384]
                    nc.scalar.activation(dst, src, EXP, bias=0.0, scale=0.125)
                    if t % 2 == 1:
                        emit_mask_batch(t // 2)
                        for i in BLOCKS_AT.get(t, []):
                            emit_block_pv(i)
                        for k in NORMS_AT.get(t, []):
                            emit_og_norm(k)

                nc.sync.dma_start(
                    out=out[h].rearrange("n p d -> p n d"),
                    in_=ot.rearrange("p (n d) -> p n d", n=NBLK),
                )
    _legalize_matmul_waits(nc)
    return nc


def make_masks() -> dict[str, np.ndarray]:
    kk = np.arange(128, dtype=np.int32)[:, None]
    qc = np.arange(384, dtype=np.int32)[None, :]
    xx = np.arange(128, dtype=np.int32)[None, :]
    return {
        "mF": (qc <= kk + 128).astype(np.float16),
        "mL": (qc >= kk + 128).astype(np.float16),
        "mO": np.stack([(xx >= kk), (xx <= kk)], axis=1).astype(np.float16),
    }


def prep_inputs(q: np.ndarray, k: np.ndarray, v: np.ndarray):
    qT = np.ascontiguousarray(
        q.reshape(B * H, L, D).transpose(0, 2, 1)).astype(np.float16)
    kT = np.ascontiguousarray(
        k.reshape(B * H, L, D).transpose(0, 2, 1)).astype(np.float16)
    qhh = np.stack([qT[:, :, 0:2176], qT[:, :, 1920:4096]], axis=1)
    khh = np.stack([kT[:, :, 0:2048], kT[:, :, 2048:4096]], axis=1)
    vb = v.reshape(B * H, NBLK, 128, D).transpose(0, 2, 1, 3)
    vAh = np.concatenate(
        [vb, np.ones((B * H, 128, NBLK, 1), np.float32)], axis=3).astype(np.float16)
    return (np.ascontiguousarray(qhh), np.ascontiguousarray(khh),
            np.ascontiguousarray(vAh))


_CACHE: dict = {}


def kernel(q: np.ndarray, k: np.ndarray, v: np.ndarray) -> np.ndarray:
    from concourse.bass_utils import run_bass_kernel_spmd

    q = np.asarray(q, dtype=np.float32)
    k = np.asarray(k, dtype=np.float32)
    v = np.asarray(v, dtype=np.float32)
    qhh, khh, vAh = prep_inputs(q, k, v)
    masks = make_masks()

    if "nc" not in _CACHE:
        _CACHE["nc"] = build_nc(HPC)
    nc = _CACHE["nc"]

    in_maps = []
    for c in range(NCORES):
        s = slice(c * HPC, (c + 1) * HPC)
        in_maps.append({"qh": qhh[s], "kh": khh[s], "vA": vAh[s], **masks})
    res = run_bass_kernel_spmd(nc, in_maps, list(range(NCORES)))
    outs = [res.results[c]["out"] for c in range(NCORES)]
    full = np.concatenate(outs, axis=0).reshape(B, H, L, D)
    return full.astype(np.float32)


if __name__ == "__main__":
    rng = np.random.default_rng(0)
    q = rng.standard_normal((B, H, L, D), dtype=np.float32)
    k = rng.standard_normal((B, H, L, D), dtype=np.float32)
    v = rng.standard_normal((B, H, L, D), dtype=np.float32)
    o = kernel(q, k, v)
    print("out", o.shape, o.dtype)
